# revision 22
# baseline (speedup 1.0000x reference)
"""GQA attention forward (B=2, S=2048, D=2048, 16 q heads / 4 kv heads, RoPE,
causal) on 8 Trainium2 NeuronCores.

Sharding: core c <-> (batch b = c//4, kv-group g = c%4). Each core computes its
4 query heads + 1 kv head end-to-end, including its row-shard of wo; the host
sums the 4 wo-partials per batch (the "all-reduce after wo" of the tensor
parallel scheme, done at gather time).

v2 layout/schedule:
  - everything stored fp16 (weights, q/k/v, es, on, wo, y partial): halves DMA
    and SBUF, speeds PE weight loads, enables 2x DVE modes; psum stays f32.
  - x is passed transposed (d-major); wq/wk columns permuted per head so RoPE
    is elementwise on partition halves (permutation cancels in q.k).
  - RoPE packed as 2 full-width muls + 2 half-width add/sub via [cos;sin] and
    [sin;cos] stacked constants.
  - scores built transposed ([t, s]); softmax denominator via ones-column
    matmul; no max-subtraction (scores are O(1), exp safe, shift-invariant).
  - the 4 heads' k=3 diagonal score tiles fused into one 512-moving matmul.
  - attention output `on` stays SBUF-resident; wo is interleaved per s-block
    (no DRAM spill, no serial tail).
  - normalization: DVE reciprocal from psum + gpsimd partition broadcast.
"""

import ml_dtypes
import numpy as np

F16 = ml_dtypes.float16 if hasattr(ml_dtypes, "float16") else np.float16
B, S, D = 2, 2048, 2048
N_HEADS, N_KV_HEADS, HD = 16, 4, 128
NH = N_HEADS // N_KV_HEADS  # q heads per core = 4
SB = 512                    # s-block (moving dim per matmul)
NSJ = S // SB               # 4 s-blocks
NT = S // HD                # 16 t-tiles (and d-tiles)
SCALE = 1.0 / np.sqrt(HD).astype(np.float32)

_PROG = None  # built once per process


def _build_program():
    import concourse.bacc as bacc
    import concourse.tile as tile
    from concourse import bass_isa, mybir

    F32 = mybir.dt.float32
    FP16 = mybir.dt.float16
    Exp = mybir.ActivationFunctionType.Exp

    nc = bacc.Bacc("TRN2", target_bir_lowering=False, debug=False)

    xt_d = nc.declare_dram_parameter("xt", [D, S], FP16, isOutput=False)
    # wqkv pre-arranged host-side as [p, m, t, c] so each m-slice DMA reads
    # 4KB contiguous per partition
    wqkv_d = nc.declare_dram_parameter(
        "wqkv", [HD, (NH + 2) * NT * HD], FP16, isOutput=False
    )
    wo_d = nc.declare_dram_parameter("wo", [NH * HD, D], FP16, isOutput=False)
    cs_d = nc.declare_dram_parameter("cs", [HD // 2, S], FP16, isOutput=False)  # cos^T
    sc_d = nc.declare_dram_parameter("sc", [HD // 2, S], FP16, isOutput=False)  # sin^T
    mask_d = nc.declare_dram_parameter("mask", [HD, NH * HD], FP16, isOutput=False)
    ident_d = nc.declare_dram_parameter("ident", [HD, HD], FP16, isOutput=False)
    ones_d = nc.declare_dram_parameter("ones", [HD, HD], FP16, isOutput=False)
    y_d = nc.declare_dram_parameter("y", [S, D], FP16, isOutput=True)

    NM = NH + 2  # 6 projection column-blocks: k, v, q0..q3
    H2 = HD // 2

    with tile.TileContext(nc) as tc:
        with (
            tc.tile_pool(name="consts", bufs=1) as consts,
            tc.tile_pool(name="persist", bufs=1) as persist,
            tc.tile_pool(name="work", bufs=2) as work,
            tc.tile_pool(name="ps", bufs=1, space="PSUM") as ps,
        ):
            # ---- small constants ----
            mask4 = consts.tile([HD, NH * HD], FP16, tag="mask4")  # tri mask x4
            ident = consts.tile([HD, HD], FP16, tag="ident")
            ones_sb = consts.tile([HD, HD], FP16, tag="ones_sb")
            ones_col = ones_sb[:, 0:1]
            cs_sb = consts.tile([HD // 2, S], FP16, tag="cs_sb")
            sc_sb = consts.tile([HD // 2, S], FP16, tag="sc_sb")

            # ---- persistent SBUF tensors ----
            kt = persist.tile([HD, S], FP16, tag="kt")            # k^T rope'd
            v_sb = persist.tile([HD, NT, HD], FP16, tag="v_sb")   # v[t, hd]
            wqkv = persist.tile([HD, NT, NM * HD], FP16, tag="wqkv")
            wo_sb = persist.tile([HD, NH, D], FP16, tag="wo_sb")
            on_all = persist.tile([HD, NH, S], FP16, tag="on_all")

            xt_r = xt_d[:, :].rearrange("(t p) s -> p t s", p=HD)
            wqkv_r = wqkv_d[:, :].rearrange("p (m t c) -> p m t c", m=NM, t=NT)

            # critical-path loads first: wqkv m-slice 0 + first x block, then
            # remaining m-slices in consumption order
            nc.scalar.dma_start(
                out=wqkv[:, :, 0:HD], in_=wqkv_r[:, 0, :, :]
            )
            sj0_quarters = []
            for ck in range(4):
                xq = work.tile(
                    [HD, NT // 4, SB], FP16, tag="xts", bufs=8, name=f"xts_0_{ck}"
                )
                nc.sync.dma_start(out=xq, in_=xt_r[:, ck * 4 : (ck + 1) * 4, 0:SB])
                sj0_quarters.append(xq)
            nc.gpsimd.dma_start(out=cs_sb, in_=cs_d[:, :])
            nc.gpsimd.dma_start(out=sc_sb, in_=sc_d[:, :])
            for m in range(1, NM):
                nc.scalar.dma_start(
                    out=wqkv[:, :, m * HD : (m + 1) * HD], in_=wqkv_r[:, m, :, :]
                )
            nc.gpsimd.dma_start(out=mask4, in_=mask_d[:, :])
            nc.gpsimd.dma_start(out=ident, in_=ident_d[:, :])
            nc.gpsimd.dma_start(out=ones_sb, in_=ones_d[:, :])
            nc.scalar.dma_start(
                out=wo_sb, in_=wo_d[:, :].rearrange("(h p) d -> p h d", p=HD)
            )

            for sj in range(NSJ):
                s0 = sj * SB
                # ---- projection of x^T[:, s-block] ----
                if sj == 0:
                    quarters = sj0_quarters
                else:
                    quarters = []
                    for ck in range(4):
                        xq = work.tile(
                            [HD, NT // 4, SB], FP16, tag="xts", bufs=8,
                            name=f"xts_{sj}_{ck}",
                        )
                        nc.sync.dma_start(
                            out=xq, in_=xt_r[:, ck * 4 : (ck + 1) * 4, s0 : s0 + SB]
                        )
                        quarters.append(xq)
                q4 = work.tile([HD, NH, SB], FP16, tag="q4", bufs=2, name=f"q4_{sj}")
                for m in range(NM):  # 0=k, 1=v, 2..5=q heads
                    pp = ps.tile([HD, SB], F32, tag="pp", bufs=2)
                    for dt in range(NT):
                        nc.tensor.matmul(
                            out=pp,
                            lhsT=wqkv[:, dt, m * HD : (m + 1) * HD],
                            rhs=quarters[dt // 4][:, dt % 4, :],
                            start=(dt == 0),
                            stop=(dt == NT - 1),
                        )
                    if m != 1:
                        # rope: rows 0:64 = even dims (xr), 64:128 = odd (xi)
                        if m == 0:
                            dst = kt[:, s0 : s0 + SB]
                        else:
                            dst = q4[:, m - 2, :]
                        c = cs_sb[:, s0 : s0 + SB]
                        sn = sc_sb[:, s0 : s0 + SB]
                        ta = work.tile([H2, SB], FP16, tag="rope_a")
                        tb = work.tile([H2, SB], FP16, tag="rope_b")
                        nc.vector.tensor_mul(out=ta, in0=pp[0:H2, :], in1=c)
                        nc.vector.tensor_mul(out=tb, in0=pp[H2:HD, :], in1=sn)
                        nc.gpsimd.tensor_sub(out=dst[0:H2, :], in0=ta, in1=tb)
                        tc2 = work.tile([H2, SB], FP16, tag="rope_a")
                        td = work.tile([H2, SB], FP16, tag="rope_b")
                        nc.vector.tensor_mul(out=tc2, in0=pp[0:H2, :], in1=sn)
                        nc.vector.tensor_mul(out=td, in0=pp[H2:HD, :], in1=c)
                        nc.gpsimd.tensor_add(out=dst[H2:HD, :], in0=tc2, in1=td)
                    else:
                        # v: psum holds v^T[hd, s-block]; transpose to v[t, hd]
                        vt_sb = work.tile([HD, SB], FP16, tag="vt_sb")
                        nc.scalar.copy(out=vt_sb, in_=pp)
                        for q in range(SB // HD):
                            pt = ps.tile([HD, HD], FP16, tag="s", bufs=3)
                            nc.tensor.transpose(
                                pt, vt_sb[:, q * HD : (q + 1) * HD], ident
                            )
                            nc.scalar.copy(out=v_sb[:, sj * 4 + q, :], in_=pt)

                # ---- fused diagonal (k=3) scores for all 4 heads ----
                ti3 = 4 * sj + 3
                ps_sd = ps.tile([HD, SB], F32, tag="s", bufs=3, name=f"sd_{sj}")
                nc.tensor.matmul(
                    out=ps_sd,
                    lhsT=kt[:, ti3 * HD : (ti3 + 1) * HD],
                    rhs=q4[:, :, 3 * HD : SB],
                    start=True,
                    stop=True,
                )
                es4d = work.tile([HD, NH * HD], FP16, tag="es4d", bufs=2)
                nc.scalar.activation(
                    out=es4d, in_=ps_sd, func=Exp, scale=float(SCALE)
                )
                nc.vector.tensor_mul(out=es4d, in0=es4d, in1=mask4)

                # ---- attention for all heads at this s-block ----
                nt = 4 * sj + 4  # causal: t-tiles 0..nt-1
                for h in range(NH):
                    ps_o = ps.tile([HD, SB], F32, tag="o", bufs=2, name=f"o_{sj}_{h}")
                    ps_den = ps.tile(
                        [1, SB], F32, tag="den", bufs=1, name=f"d_{sj}_{h}"
                    )
                    for ti in range(nt - 1):
                        k = ti - 4 * sj
                        c0 = max(0, k) * HD  # first valid column (diag band)
                        ps_s = ps.tile(
                            [HD, SB], F32, tag="s", bufs=3, name=f"s_{sj}_{h}_{ti}"
                        )
                        nc.tensor.matmul(
                            out=ps_s[:, c0:SB],
                            lhsT=kt[:, ti * HD : (ti + 1) * HD],
                            rhs=q4[:, h, c0:SB],
                            start=True,
                            stop=True,
                        )
                        es = work.tile([HD, SB], FP16, tag="es", bufs=4)
                        nc.scalar.activation(
                            out=es[:, c0:SB], in_=ps_s[:, c0:SB], func=Exp,
                            scale=float(SCALE),
                        )
                        if k >= 0:
                            # triangular part: first 128 valid columns
                            nc.vector.tensor_mul(
                                out=es[:, c0 : c0 + HD],
                                in0=es[:, c0 : c0 + HD],
                                in1=mask4[:, 0:HD],
                            )
                        nc.tensor.matmul(
                            out=ps_o[:, c0:SB],
                            lhsT=v_sb[:, ti, :],
                            rhs=es[:, c0:SB],
                            start=(ti == 0),
                            stop=False,
                        )
                        nc.tensor.matmul(
                            out=ps_den[:, c0:SB],
                            lhsT=ones_col,
                            rhs=es[:, c0:SB],
                            start=(ti == 0),
                            stop=False,
                        )
                    # diagonal tile from the fused block
                    c3 = 3 * HD
                    nc.tensor.matmul(
                        out=ps_o[:, c3:SB],
                        lhsT=v_sb[:, ti3, :],
                        rhs=es4d[:, h * HD : (h + 1) * HD],
                        start=False,
                        stop=True,
                    )
                    nc.tensor.matmul(
                        out=ps_den[:, c3:SB],
                        lhsT=ones_col,
                        rhs=es4d[:, h * HD : (h + 1) * HD],
                        start=False,
                        stop=True,
                    )
                    # normalize: on = ps_o * (1/den) broadcast over partitions
                    rb1 = work.tile([1, SB], F32, tag="rb1")
                    nc.vector.reciprocal_approx_fast(out=rb1, in_=ps_den)
                    db = work.tile([HD, SB], F32, tag="db")
                    nc.gpsimd.partition_broadcast(db, rb1, channels=HD)
                    nc.vector.tensor_mul(
                        out=on_all[:, h, s0 : s0 + SB], in0=ps_o, in1=db
                    )

                # ---- wo for this s-block: y[t, :] = sum_h on_h^T @ wo_h ----
                # wo psum shares the "o" ring (attention AV is done for sj)
                for st in range(4):
                    t0 = s0 + st * HD
                    y_sb = work.tile([HD, NSJ, SB], FP16, tag="y_sb", bufs=2)
                    for dj in range(NSJ):
                        ps_y = ps.tile(
                            [HD, SB], F32, tag="o", bufs=2, name=f"y_{sj}_{st}_{dj}"
                        )
                        for hh in range(NH):
                            nc.tensor.matmul(
                                out=ps_y,
                                lhsT=on_all[:, hh, t0 : t0 + HD],
                                rhs=wo_sb[:, hh, dj * SB : (dj + 1) * SB],
                                start=(hh == 0),
                                stop=(hh == NH - 1),
                            )
                        nc.scalar.copy(out=y_sb[:, dj, :], in_=ps_y)
                    nc.sync.dma_start(out=y_d[t0 : t0 + HD, :], in_=y_sb)

    nc.compile()
    return nc


def _get_program():
    global _PROG
    if _PROG is None:
        _PROG = _build_program()
    return _PROG


def _make_in_maps(x, freqs_cos, freqs_sin, wq, wk, wv, wo):
    perm = np.concatenate([np.arange(0, HD, 2), np.arange(1, HD, 2)])  # even|odd

    cs = np.ascontiguousarray(np.asarray(freqs_cos, np.float32).T).astype(F16)
    sc = np.ascontiguousarray(np.asarray(freqs_sin, np.float32).T).astype(F16)

    # triangular causal mask, replicated x4 for the fused diagonal block
    tt = np.arange(HD)[:, None]
    ss = np.arange(HD)[None, :]
    tri = (tt <= ss).astype(np.float32)  # [128, 128]
    mask4 = np.tile(tri, (1, NH)).astype(F16)  # [128, 4*128]
    ident = np.eye(HD, dtype=np.float32).astype(F16)
    ones = np.ones((HD, HD), dtype=np.float32).astype(F16)

    # permute q/k head-dim columns so rope pairs land on partition halves
    def permute_heads(w, n_heads):
        w = w.reshape(D, n_heads, HD)
        return w[:, :, perm].reshape(D, n_heads * HD)

    wq_p = permute_heads(np.asarray(wq, np.float32), N_HEADS)
    wk_p = permute_heads(np.asarray(wk, np.float32), N_KV_HEADS)
    wv_ = np.asarray(wv, np.float32)
    wo_ = np.asarray(wo, np.float32)

    xt16 = [
        np.ascontiguousarray(np.asarray(x, np.float32)[b].T).astype(F16)
        for b in range(B)
    ]

    in_maps = []
    for c in range(8):
        b, g = divmod(c, 4)
        wqkv = np.concatenate(
            [
                wk_p[:, g * HD : (g + 1) * HD],
                wv_[:, g * HD : (g + 1) * HD],
                wq_p[:, g * NH * HD : (g + 1) * NH * HD],
            ],
            axis=1,
        )
        # rearrange to [p, m, t, c] so each m-slice is 4KB/partition contiguous
        wqkv = (
            wqkv.reshape(NT, HD, NH + 2, HD)
            .transpose(1, 2, 0, 3)
            .reshape(HD, (NH + 2) * NT * HD)
        )
        wqkv = np.ascontiguousarray(wqkv).astype(F16)
        in_maps.append(
            {
                "xt": xt16[b],
                "wqkv": wqkv,
                "wo": np.ascontiguousarray(
                    wo_[g * NH * HD : (g + 1) * NH * HD, :]
                ).astype(F16),
                "cs": cs,
                "sc": sc,
                "mask": mask4,
                "ident": ident,
                "ones": ones,
            }
        )
    return in_maps


def run(x, freqs_cos, freqs_sin, wq, wk, wv, wo, trace=False):
    from concourse.bass_utils import run_bass_kernel_spmd

    nc = _get_program()
    in_maps = _make_in_maps(x, freqs_cos, freqs_sin, wq, wk, wv, wo)
    res = run_bass_kernel_spmd(nc, in_maps, list(range(8)), trace=trace)
    out = np.empty((B, S, D), dtype=np.float32)
    for b in range(B):
        acc = res.results[b * 4]["y"].astype(np.float32)
        for g in range(1, 4):
            acc += res.results[b * 4 + g]["y"].astype(np.float32)
        out[b] = acc
    return out, res


def kernel(x, freqs_cos, freqs_sin, wq, wk, wv, wo):
    out, _ = run(x, freqs_cos, freqs_sin, wq, wk, wv, wo, trace=False)
    return out


# revision 26
# speedup vs baseline: 1.1654x; 1.1654x over previous
"""GQA attention forward (B=2, S=2048, D=2048, 16 q heads / 4 kv heads, RoPE,
causal) on 8 Trainium2 NeuronCores.

Sharding: core c <-> (batch b = c//4, kv-group g = c%4). Each core computes its
4 query heads + 1 kv head end-to-end, including its row-shard of wo; the host
sums the 4 wo-partials per batch (the "all-reduce after wo" of the tensor
parallel scheme, done at gather time).

v2 layout/schedule:
  - everything stored fp16 (weights, q/k/v, es, on, wo, y partial): halves DMA
    and SBUF, speeds PE weight loads, enables 2x DVE modes; psum stays f32.
  - x is passed transposed (d-major); wq/wk columns permuted per head so RoPE
    is elementwise on partition halves (permutation cancels in q.k).
  - RoPE packed as 2 full-width muls + 2 half-width add/sub via [cos;sin] and
    [sin;cos] stacked constants.
  - scores built transposed ([t, s]); softmax denominator via ones-column
    matmul; no max-subtraction (scores are O(1), exp safe, shift-invariant).
  - the 4 heads' k=3 diagonal score tiles fused into one 512-moving matmul.
  - attention output `on` stays SBUF-resident; wo is interleaved per s-block
    (no DRAM spill, no serial tail).
  - normalization: DVE reciprocal from psum + gpsimd partition broadcast.
"""

import ml_dtypes
import numpy as np

F16 = ml_dtypes.float16 if hasattr(ml_dtypes, "float16") else np.float16
B, S, D = 2, 2048, 2048
N_HEADS, N_KV_HEADS, HD = 16, 4, 128
NH = N_HEADS // N_KV_HEADS  # q heads per core = 4
SB = 512                    # s-block (moving dim per matmul)
NSJ = S // SB               # 4 s-blocks
NT = S // HD                # 16 t-tiles (and d-tiles)
SCALE = 1.0 / np.sqrt(HD).astype(np.float32)

_PROG = None  # built once per process


def _build_program():
    import concourse.bacc as bacc
    import concourse.tile as tile
    from concourse import bass_isa, mybir

    F32 = mybir.dt.float32
    FP16 = mybir.dt.float16
    Exp = mybir.ActivationFunctionType.Exp

    nc = bacc.Bacc("TRN2", target_bir_lowering=False, debug=False)

    xt_d = nc.declare_dram_parameter("xt", [D, S], FP16, isOutput=False)
    # wqkv pre-arranged host-side as [p, m, t, c] so each m-slice DMA reads
    # 4KB contiguous per partition
    wqkv_d = nc.declare_dram_parameter(
        "wqkv", [HD, (NH + 2) * NT * HD], FP16, isOutput=False
    )
    wo_d = nc.declare_dram_parameter("wo", [NH * HD, D], FP16, isOutput=False)
    cs_d = nc.declare_dram_parameter("cs", [HD // 2, S], FP16, isOutput=False)  # cos^T
    sc_d = nc.declare_dram_parameter("sc", [HD // 2, S], FP16, isOutput=False)  # sin^T
    mask_d = nc.declare_dram_parameter("mask", [HD, NH * HD], FP16, isOutput=False)
    ident_d = nc.declare_dram_parameter("ident", [HD, HD], FP16, isOutput=False)
    ones_d = nc.declare_dram_parameter("ones", [HD, HD], FP16, isOutput=False)
    y_d = nc.declare_dram_parameter("y", [S, D], FP16, isOutput=True)

    NM = NH + 2  # 6 projection column-blocks: k, v, q0..q3
    H2 = HD // 2

    with tile.TileContext(nc) as tc:
        with (
            tc.tile_pool(name="consts", bufs=1) as consts,
            tc.tile_pool(name="persist", bufs=1) as persist,
            tc.tile_pool(name="work", bufs=2) as work,
            tc.tile_pool(name="ps", bufs=1, space="PSUM") as ps,
        ):
            # ---- small constants ----
            mask4 = consts.tile([HD, NH * HD], FP16, tag="mask4")  # tri mask x4
            ident = consts.tile([HD, HD], FP16, tag="ident")
            ones_sb = consts.tile([HD, HD], FP16, tag="ones_sb")
            ones_col = ones_sb[:, 0:1]
            cs_sb = consts.tile([HD // 2, S], FP16, tag="cs_sb")
            sc_sb = consts.tile([HD // 2, S], FP16, tag="sc_sb")

            # ---- persistent SBUF tensors ----
            kt = persist.tile([HD, S], FP16, tag="kt")            # k^T rope'd
            v_sb = persist.tile([HD, NT, HD], FP16, tag="v_sb")   # v[t, hd]
            wqkv = persist.tile([HD, NT, NM * HD], FP16, tag="wqkv")
            wo_sb = persist.tile([HD, NH, D], FP16, tag="wo_sb")
            on_all = persist.tile([HD, NH, S], FP16, tag="on_all")

            xt_r = xt_d[:, :].rearrange("(t p) s -> p t s", p=HD)
            wqkv_r = wqkv_d[:, :].rearrange("p (m t c) -> p m t c", m=NM, t=NT)

            # critical-path loads first: wqkv m-slice 0 + first x block, then
            # remaining m-slices in consumption order
            nc.scalar.dma_start(
                out=wqkv[:, :, 0:HD], in_=wqkv_r[:, 0, :, :]
            )
            sj0_quarters = []
            for ck in range(4):
                xq = work.tile(
                    [HD, NT // 4, SB], FP16, tag="xts", bufs=8, name=f"xts_0_{ck}"
                )
                nc.sync.dma_start(out=xq, in_=xt_r[:, ck * 4 : (ck + 1) * 4, 0:SB])
                sj0_quarters.append(xq)
            nc.gpsimd.dma_start(out=cs_sb, in_=cs_d[:, :])
            nc.gpsimd.dma_start(out=sc_sb, in_=sc_d[:, :])
            for m in range(1, NM):
                nc.scalar.dma_start(
                    out=wqkv[:, :, m * HD : (m + 1) * HD], in_=wqkv_r[:, m, :, :]
                )
            nc.gpsimd.dma_start(out=mask4, in_=mask_d[:, :])
            nc.gpsimd.dma_start(out=ident, in_=ident_d[:, :])
            nc.gpsimd.dma_start(out=ones_sb, in_=ones_d[:, :])
            nc.scalar.dma_start(
                out=wo_sb, in_=wo_d[:, :].rearrange("(h p) d -> p h d", p=HD)
            )

            for sj in range(NSJ):
                s0 = sj * SB
                # ---- projection of x^T[:, s-block] ----
                if sj == 0:
                    quarters = sj0_quarters
                else:
                    quarters = []
                    for ck in range(4):
                        xq = work.tile(
                            [HD, NT // 4, SB], FP16, tag="xts", bufs=8,
                            name=f"xts_{sj}_{ck}",
                        )
                        nc.sync.dma_start(
                            out=xq, in_=xt_r[:, ck * 4 : (ck + 1) * 4, s0 : s0 + SB]
                        )
                        quarters.append(xq)
                q4 = work.tile([HD, NH, SB], FP16, tag="q4", bufs=2, name=f"q4_{sj}")
                for m in range(NM):  # 0=k, 1=v, 2..5=q heads
                    pp = ps.tile([HD, SB], F32, tag="pp", bufs=2)
                    for dt in range(NT):
                        nc.tensor.matmul(
                            out=pp,
                            lhsT=wqkv[:, dt, m * HD : (m + 1) * HD],
                            rhs=quarters[dt // 4][:, dt % 4, :],
                            start=(dt == 0),
                            stop=(dt == NT - 1),
                        )
                    if m != 1:
                        # rope: rows 0:64 = even dims (xr), 64:128 = odd (xi)
                        if m == 0:
                            dst = kt[:, s0 : s0 + SB]
                        else:
                            dst = q4[:, m - 2, :]
                        c = cs_sb[:, s0 : s0 + SB]
                        sn = sc_sb[:, s0 : s0 + SB]
                        ta = work.tile([H2, SB], FP16, tag="rope_a")
                        tb = work.tile([H2, SB], FP16, tag="rope_b")
                        nc.vector.tensor_mul(out=ta, in0=pp[0:H2, :], in1=c)
                        nc.vector.tensor_mul(out=tb, in0=pp[H2:HD, :], in1=sn)
                        nc.vector.tensor_sub(out=dst[0:H2, :], in0=ta, in1=tb)
                        tc2 = work.tile([H2, SB], FP16, tag="rope_a")
                        td = work.tile([H2, SB], FP16, tag="rope_b")
                        nc.vector.tensor_mul(out=tc2, in0=pp[0:H2, :], in1=sn)
                        nc.vector.tensor_mul(out=td, in0=pp[H2:HD, :], in1=c)
                        nc.vector.tensor_add(out=dst[H2:HD, :], in0=tc2, in1=td)
                    else:
                        # v: psum holds v^T[hd, s-block]; transpose to v[t, hd]
                        vt_sb = work.tile([HD, SB], FP16, tag="vt_sb")
                        nc.scalar.copy(out=vt_sb, in_=pp)
                        for q in range(SB // HD):
                            pt = ps.tile([HD, HD], FP16, tag="s", bufs=3)
                            nc.tensor.transpose(
                                pt, vt_sb[:, q * HD : (q + 1) * HD], ident
                            )
                            nc.scalar.copy(out=v_sb[:, sj * 4 + q, :], in_=pt)

                # ---- fused diagonal (k=3) scores for all 4 heads ----
                ti3 = 4 * sj + 3
                ps_sd = ps.tile([HD, SB], F32, tag="s", bufs=3, name=f"sd_{sj}")
                nc.tensor.matmul(
                    out=ps_sd,
                    lhsT=kt[:, ti3 * HD : (ti3 + 1) * HD],
                    rhs=q4[:, :, 3 * HD : SB],
                    start=True,
                    stop=True,
                )
                es4d = work.tile([HD, NH * HD], FP16, tag="es4d", bufs=2)
                nc.scalar.activation(
                    out=es4d, in_=ps_sd, func=Exp, scale=float(SCALE)
                )
                nc.vector.tensor_mul(out=es4d, in0=es4d, in1=mask4)

                # ---- attention for all heads at this s-block ----
                nt = 4 * sj + 4  # causal: t-tiles 0..nt-1
                for h in range(NH):
                    ps_o = ps.tile([HD, SB], F32, tag="o", bufs=2, name=f"o_{sj}_{h}")
                    ps_den = ps.tile(
                        [1, SB], F32, tag="den", bufs=1, name=f"d_{sj}_{h}"
                    )
                    for ti in range(nt - 1):
                        k = ti - 4 * sj
                        c0 = max(0, k) * HD  # first valid column (diag band)
                        ps_s = ps.tile(
                            [HD, SB], F32, tag="s", bufs=3, name=f"s_{sj}_{h}_{ti}"
                        )
                        nc.tensor.matmul(
                            out=ps_s[:, c0:SB],
                            lhsT=kt[:, ti * HD : (ti + 1) * HD],
                            rhs=q4[:, h, c0:SB],
                            start=True,
                            stop=True,
                        )
                        es = work.tile([HD, SB], FP16, tag="es", bufs=4)
                        nc.scalar.activation(
                            out=es[:, c0:SB], in_=ps_s[:, c0:SB], func=Exp,
                            scale=float(SCALE),
                        )
                        if k >= 0:
                            # triangular part: first 128 valid columns
                            nc.vector.tensor_mul(
                                out=es[:, c0 : c0 + HD],
                                in0=es[:, c0 : c0 + HD],
                                in1=mask4[:, 0:HD],
                            )
                        nc.tensor.matmul(
                            out=ps_o[:, c0:SB],
                            lhsT=v_sb[:, ti, :],
                            rhs=es[:, c0:SB],
                            start=(ti == 0),
                            stop=False,
                        )
                        nc.tensor.matmul(
                            out=ps_den[:, c0:SB],
                            lhsT=ones_col,
                            rhs=es[:, c0:SB],
                            start=(ti == 0),
                            stop=False,
                        )
                    # diagonal tile from the fused block
                    c3 = 3 * HD
                    nc.tensor.matmul(
                        out=ps_o[:, c3:SB],
                        lhsT=v_sb[:, ti3, :],
                        rhs=es4d[:, h * HD : (h + 1) * HD],
                        start=False,
                        stop=True,
                    )
                    nc.tensor.matmul(
                        out=ps_den[:, c3:SB],
                        lhsT=ones_col,
                        rhs=es4d[:, h * HD : (h + 1) * HD],
                        start=False,
                        stop=True,
                    )
                    # normalize: on = ps_o * (1/den) broadcast over partitions
                    rb1 = work.tile([1, SB], F32, tag="rb1")
                    nc.vector.reciprocal_approx_fast(out=rb1, in_=ps_den)
                    db = work.tile([HD, SB], F32, tag="db")
                    nc.gpsimd.partition_broadcast(db, rb1, channels=HD)
                    nc.vector.tensor_mul(
                        out=on_all[:, h, s0 : s0 + SB], in0=ps_o, in1=db
                    )

                # ---- wo for this s-block: y[t, :] = sum_h on_h^T @ wo_h ----
                # wo psum shares the "o" ring (attention AV is done for sj)
                for st in range(4):
                    t0 = s0 + st * HD
                    y_sb = work.tile([HD, NSJ, SB], FP16, tag="y_sb", bufs=2)
                    for dj in range(NSJ):
                        ps_y = ps.tile(
                            [HD, SB], F32, tag="o", bufs=2, name=f"y_{sj}_{st}_{dj}"
                        )
                        for hh in range(NH):
                            nc.tensor.matmul(
                                out=ps_y,
                                lhsT=on_all[:, hh, t0 : t0 + HD],
                                rhs=wo_sb[:, hh, dj * SB : (dj + 1) * SB],
                                start=(hh == 0),
                                stop=(hh == NH - 1),
                            )
                        if dj % 2 == 0:
                            nc.scalar.copy(out=y_sb[:, dj, :], in_=ps_y)
                        else:
                            nc.vector.tensor_copy(out=y_sb[:, dj, :], in_=ps_y)
                    nc.sync.dma_start(out=y_d[t0 : t0 + HD, :], in_=y_sb)

    nc.compile()
    return nc


def _get_program():
    global _PROG
    if _PROG is None:
        _PROG = _build_program()
    return _PROG


def _make_in_maps(x, freqs_cos, freqs_sin, wq, wk, wv, wo):
    perm = np.concatenate([np.arange(0, HD, 2), np.arange(1, HD, 2)])  # even|odd

    cs = np.ascontiguousarray(np.asarray(freqs_cos, np.float32).T).astype(F16)
    sc = np.ascontiguousarray(np.asarray(freqs_sin, np.float32).T).astype(F16)

    # triangular causal mask, replicated x4 for the fused diagonal block
    tt = np.arange(HD)[:, None]
    ss = np.arange(HD)[None, :]
    tri = (tt <= ss).astype(np.float32)  # [128, 128]
    mask4 = np.tile(tri, (1, NH)).astype(F16)  # [128, 4*128]
    ident = np.eye(HD, dtype=np.float32).astype(F16)
    ones = np.ones((HD, HD), dtype=np.float32).astype(F16)

    # permute q/k head-dim columns so rope pairs land on partition halves
    def permute_heads(w, n_heads):
        w = w.reshape(D, n_heads, HD)
        return w[:, :, perm].reshape(D, n_heads * HD)

    wq_p = permute_heads(np.asarray(wq, np.float32), N_HEADS)
    wk_p = permute_heads(np.asarray(wk, np.float32), N_KV_HEADS)
    wv_ = np.asarray(wv, np.float32)
    wo_ = np.asarray(wo, np.float32)

    xt16 = [
        np.ascontiguousarray(np.asarray(x, np.float32)[b].T).astype(F16)
        for b in range(B)
    ]

    in_maps = []
    for c in range(8):
        b, g = divmod(c, 4)
        wqkv = np.concatenate(
            [
                wk_p[:, g * HD : (g + 1) * HD],
                wv_[:, g * HD : (g + 1) * HD],
                wq_p[:, g * NH * HD : (g + 1) * NH * HD],
            ],
            axis=1,
        )
        # rearrange to [p, m, t, c] so each m-slice is 4KB/partition contiguous
        wqkv = (
            wqkv.reshape(NT, HD, NH + 2, HD)
            .transpose(1, 2, 0, 3)
            .reshape(HD, (NH + 2) * NT * HD)
        )
        wqkv = np.ascontiguousarray(wqkv).astype(F16)
        in_maps.append(
            {
                "xt": xt16[b],
                "wqkv": wqkv,
                "wo": np.ascontiguousarray(
                    wo_[g * NH * HD : (g + 1) * NH * HD, :]
                ).astype(F16),
                "cs": cs,
                "sc": sc,
                "mask": mask4,
                "ident": ident,
                "ones": ones,
            }
        )
    return in_maps


def run(x, freqs_cos, freqs_sin, wq, wk, wv, wo, trace=False):
    from concourse.bass_utils import run_bass_kernel_spmd

    nc = _get_program()
    in_maps = _make_in_maps(x, freqs_cos, freqs_sin, wq, wk, wv, wo)
    res = run_bass_kernel_spmd(nc, in_maps, list(range(8)), trace=trace)
    out = np.empty((B, S, D), dtype=np.float32)
    for b in range(B):
        acc = res.results[b * 4]["y"].astype(np.float32)
        for g in range(1, 4):
            acc += res.results[b * 4 + g]["y"].astype(np.float32)
        out[b] = acc
    return out, res


def kernel(x, freqs_cos, freqs_sin, wq, wk, wv, wo):
    out, _ = run(x, freqs_cos, freqs_sin, wq, wk, wv, wo, trace=False)
    return out


# revision 27
# speedup vs baseline: 1.1989x; 1.0288x over previous
"""GQA attention forward (B=2, S=2048, D=2048, 16 q heads / 4 kv heads, RoPE,
causal) on 8 Trainium2 NeuronCores.

Sharding: core c <-> (batch b = c//4, kv-group g = c%4). Each core computes its
4 query heads + 1 kv head end-to-end, including its row-shard of wo; the host
sums the 4 wo-partials per batch (the "all-reduce after wo" of the tensor
parallel scheme, done at gather time).

v2 layout/schedule:
  - everything stored fp16 (weights, q/k/v, es, on, wo, y partial): halves DMA
    and SBUF, speeds PE weight loads, enables 2x DVE modes; psum stays f32.
  - x is passed transposed (d-major); wq/wk columns permuted per head so RoPE
    is elementwise on partition halves (permutation cancels in q.k).
  - RoPE packed as 2 full-width muls + 2 half-width add/sub via [cos;sin] and
    [sin;cos] stacked constants.
  - scores built transposed ([t, s]); softmax denominator via ones-column
    matmul; no max-subtraction (scores are O(1), exp safe, shift-invariant).
  - the 4 heads' k=3 diagonal score tiles fused into one 512-moving matmul.
  - attention output `on` stays SBUF-resident; wo is interleaved per s-block
    (no DRAM spill, no serial tail).
  - normalization: DVE reciprocal from psum + gpsimd partition broadcast.
"""

import ml_dtypes
import numpy as np

F16 = ml_dtypes.float16 if hasattr(ml_dtypes, "float16") else np.float16
B, S, D = 2, 2048, 2048
N_HEADS, N_KV_HEADS, HD = 16, 4, 128
NH = N_HEADS // N_KV_HEADS  # q heads per core = 4
SB = 512                    # s-block (moving dim per matmul)
NSJ = S // SB               # 4 s-blocks
NT = S // HD                # 16 t-tiles (and d-tiles)
SCALE = 1.0 / np.sqrt(HD).astype(np.float32)

_PROG = None  # built once per process


def _build_program():
    import concourse.bacc as bacc
    import concourse.tile as tile
    from concourse import bass_isa, mybir

    F32 = mybir.dt.float32
    FP16 = mybir.dt.float16
    Exp = mybir.ActivationFunctionType.Exp

    nc = bacc.Bacc("TRN2", target_bir_lowering=False, debug=False)

    xt_d = nc.declare_dram_parameter("xt", [D, S], FP16, isOutput=False)
    # wqkv pre-arranged host-side as [p, m, t, c] so each m-slice DMA reads
    # 4KB contiguous per partition
    wqkv_d = nc.declare_dram_parameter(
        "wqkv", [HD, (NH + 2) * NT * HD], FP16, isOutput=False
    )
    wo_d = nc.declare_dram_parameter("wo", [NH * HD, D], FP16, isOutput=False)
    cs_d = nc.declare_dram_parameter("cs", [HD // 2, S], FP16, isOutput=False)  # cos^T
    sc_d = nc.declare_dram_parameter("sc", [HD // 2, S], FP16, isOutput=False)  # sin^T
    mask_d = nc.declare_dram_parameter("mask", [HD, NH * HD], FP16, isOutput=False)
    ident_d = nc.declare_dram_parameter("ident", [HD, HD], FP16, isOutput=False)
    ones_d = nc.declare_dram_parameter("ones", [HD, HD], FP16, isOutput=False)
    y_d = nc.declare_dram_parameter("y", [S, D], FP16, isOutput=True)

    NM = NH + 2  # 6 projection column-blocks: k, v, q0..q3
    H2 = HD // 2

    with tile.TileContext(nc) as tc:
        with (
            tc.tile_pool(name="consts", bufs=1) as consts,
            tc.tile_pool(name="persist", bufs=1) as persist,
            tc.tile_pool(name="work", bufs=2) as work,
            tc.tile_pool(name="ps", bufs=1, space="PSUM") as ps,
        ):
            # ---- small constants ----
            mask4 = consts.tile([HD, NH * HD], FP16, tag="mask4")  # tri mask x4
            ident = consts.tile([HD, HD], FP16, tag="ident")
            ones_sb = consts.tile([HD, HD], FP16, tag="ones_sb")
            ones_col = ones_sb[:, 0:1]
            cs_sb = consts.tile([HD // 2, S], FP16, tag="cs_sb")
            sc_sb = consts.tile([HD // 2, S], FP16, tag="sc_sb")

            # ---- persistent SBUF tensors ----
            kt = persist.tile([HD, S], FP16, tag="kt")            # k^T rope'd
            v_sb = persist.tile([HD, NT, HD], FP16, tag="v_sb")   # v[t, hd]
            wqkv = persist.tile([HD, NT, NM * HD], FP16, tag="wqkv")
            wo_sb = persist.tile([HD, NH, D], FP16, tag="wo_sb")
            on_all = persist.tile([HD, NH, S], FP16, tag="on_all")

            xt_r = xt_d[:, :].rearrange("(t p) s -> p t s", p=HD)
            wqkv_r = wqkv_d[:, :].rearrange("p (m t c) -> p m t c", m=NM, t=NT)

            # critical-path loads first: wqkv m-slice 0 + first x block, then
            # remaining m-slices in consumption order
            nc.scalar.dma_start(
                out=wqkv[:, :, 0:HD], in_=wqkv_r[:, 0, :, :]
            )
            sj0_quarters = []
            for ck in range(4):
                xq = work.tile(
                    [HD, NT // 4, SB], FP16, tag="xts", bufs=8, name=f"xts_0_{ck}"
                )
                nc.sync.dma_start(out=xq, in_=xt_r[:, ck * 4 : (ck + 1) * 4, 0:SB])
                sj0_quarters.append(xq)
            nc.gpsimd.dma_start(out=cs_sb, in_=cs_d[:, :])
            nc.gpsimd.dma_start(out=sc_sb, in_=sc_d[:, :])
            for m in range(1, NM):
                nc.scalar.dma_start(
                    out=wqkv[:, :, m * HD : (m + 1) * HD], in_=wqkv_r[:, m, :, :]
                )
            nc.gpsimd.dma_start(out=mask4, in_=mask_d[:, :])
            nc.gpsimd.dma_start(out=ident, in_=ident_d[:, :])
            nc.gpsimd.dma_start(out=ones_sb, in_=ones_d[:, :])
            nc.scalar.dma_start(
                out=wo_sb, in_=wo_d[:, :].rearrange("(h p) d -> p h d", p=HD)
            )

            for sj in range(NSJ):
                s0 = sj * SB
                # ---- projection of x^T[:, s-block] ----
                if sj == 0:
                    quarters = sj0_quarters
                else:
                    quarters = []
                    for ck in range(4):
                        xq = work.tile(
                            [HD, NT // 4, SB], FP16, tag="xts", bufs=8,
                            name=f"xts_{sj}_{ck}",
                        )
                        nc.sync.dma_start(
                            out=xq, in_=xt_r[:, ck * 4 : (ck + 1) * 4, s0 : s0 + SB]
                        )
                        quarters.append(xq)
                q4 = work.tile([HD, NH, SB], FP16, tag="q4", bufs=2, name=f"q4_{sj}")
                for m in range(NM):  # 0=k, 1=v, 2..5=q heads
                    pp = ps.tile([HD, SB], F32, tag="pp", bufs=2)
                    for dt in range(NT):
                        nc.tensor.matmul(
                            out=pp,
                            lhsT=wqkv[:, dt, m * HD : (m + 1) * HD],
                            rhs=quarters[dt // 4][:, dt % 4, :],
                            start=(dt == 0),
                            stop=(dt == NT - 1),
                        )
                    if m != 1:
                        # rope: rows 0:64 = even dims (xr), 64:128 = odd (xi)
                        if m == 0:
                            dst = kt[:, s0 : s0 + SB]
                        else:
                            dst = q4[:, m - 2, :]
                        c = cs_sb[:, s0 : s0 + SB]
                        sn = sc_sb[:, s0 : s0 + SB]
                        ta = work.tile([H2, SB], FP16, tag="rope_a")
                        tb = work.tile([H2, SB], FP16, tag="rope_b")
                        nc.vector.tensor_mul(out=ta, in0=pp[0:H2, :], in1=c)
                        nc.vector.tensor_mul(out=tb, in0=pp[H2:HD, :], in1=sn)
                        nc.vector.tensor_sub(out=dst[0:H2, :], in0=ta, in1=tb)
                        tc2 = work.tile([H2, SB], FP16, tag="rope_a")
                        td = work.tile([H2, SB], FP16, tag="rope_b")
                        nc.vector.tensor_mul(out=tc2, in0=pp[0:H2, :], in1=sn)
                        nc.vector.tensor_mul(out=td, in0=pp[H2:HD, :], in1=c)
                        nc.vector.tensor_add(out=dst[H2:HD, :], in0=tc2, in1=td)
                    else:
                        # v: psum holds v^T[hd, s-block]; transpose to v[t, hd]
                        vt_sb = work.tile([HD, SB], FP16, tag="vt_sb")
                        nc.scalar.copy(out=vt_sb, in_=pp)
                        for q in range(SB // HD):
                            pt = ps.tile([HD, HD], FP16, tag="s", bufs=3)
                            nc.tensor.transpose(
                                pt, vt_sb[:, q * HD : (q + 1) * HD], ident
                            )
                            nc.scalar.copy(out=v_sb[:, sj * 4 + q, :], in_=pt)

                # ---- fused diagonal (k=3) scores for all 4 heads ----
                ti3 = 4 * sj + 3
                ps_sd = ps.tile([HD, SB], F32, tag="s", bufs=3, name=f"sd_{sj}")
                nc.tensor.matmul(
                    out=ps_sd,
                    lhsT=kt[:, ti3 * HD : (ti3 + 1) * HD],
                    rhs=q4[:, :, 3 * HD : SB],
                    start=True,
                    stop=True,
                )
                es4d = work.tile([HD, NH * HD], FP16, tag="es4d", bufs=2)
                nc.scalar.activation(
                    out=es4d, in_=ps_sd, func=Exp, scale=float(SCALE)
                )
                nc.vector.tensor_mul(out=es4d, in0=es4d, in1=mask4)

                # ---- attention for all heads at this s-block ----
                # softmax denominator: es tiles are pair-summed on the DVE and
                # the ones-matmul runs once per pair (halves the PE den rows)
                nt = 4 * sj + 4  # causal: t-tiles 0..nt-1
                for h in range(NH):
                    ps_o = ps.tile([HD, SB], F32, tag="o", bufs=2, name=f"o_{sj}_{h}")
                    ps_den = ps.tile(
                        [1, SB], F32, tag="den", bufs=1, name=f"d_{sj}_{h}"
                    )
                    nfull = 4 * sj  # full tiles (k < 0)
                    npairs = nfull // 2 + 2  # + 2 diagonal pairs
                    pair_idx = 0
                    es_prev = None
                    for ti in range(nt - 1):
                        k = ti - 4 * sj
                        c0 = max(0, k) * HD  # first valid column (diag band)
                        ps_s = ps.tile(
                            [HD, SB], F32, tag="s", bufs=3, name=f"s_{sj}_{h}_{ti}"
                        )
                        nc.tensor.matmul(
                            out=ps_s[:, c0:SB],
                            lhsT=kt[:, ti * HD : (ti + 1) * HD],
                            rhs=q4[:, h, c0:SB],
                            start=True,
                            stop=True,
                        )
                        es = work.tile([HD, SB], FP16, tag="es", bufs=4)
                        nc.scalar.activation(
                            out=es[:, c0:SB], in_=ps_s[:, c0:SB], func=Exp,
                            scale=float(SCALE),
                        )
                        if k >= 0:
                            # triangular part: first 128 valid columns
                            nc.vector.tensor_mul(
                                out=es[:, c0 : c0 + HD],
                                in0=es[:, c0 : c0 + HD],
                                in1=mask4[:, 0:HD],
                            )
                        nc.tensor.matmul(
                            out=ps_o[:, c0:SB],
                            lhsT=v_sb[:, ti, :],
                            rhs=es[:, c0:SB],
                            start=(ti == 0),
                            stop=False,
                        )
                        if k < 0 or k == 2:
                            if es_prev is None:
                                es_prev = es
                            else:
                                # full-tile pair: sum into even tile, one den mm
                                nc.vector.tensor_add(
                                    out=es_prev, in0=es_prev, in1=es
                                )
                                nc.tensor.matmul(
                                    out=ps_den,
                                    lhsT=ones_col,
                                    rhs=es_prev,
                                    start=(pair_idx == 0),
                                    stop=False,
                                )
                                pair_idx += 1
                                es_prev = None
                        elif k == 0:
                            es_k0 = es
                        elif k == 1:
                            # diag pair (k0,k1): k1 covers cols 128:512
                            nc.vector.tensor_add(
                                out=es_k0[:, HD:SB],
                                in0=es_k0[:, HD:SB],
                                in1=es[:, HD:SB],
                            )
                            nc.tensor.matmul(
                                out=ps_den,
                                lhsT=ones_col,
                                rhs=es_k0,
                                start=(pair_idx == 0),
                                stop=False,
                            )
                            pair_idx += 1
                    # diagonal tile k=3 from the fused block
                    c3 = 3 * HD
                    es4d_h = es4d[:, h * HD : (h + 1) * HD]
                    nc.tensor.matmul(
                        out=ps_o[:, c3:SB],
                        lhsT=v_sb[:, ti3, :],
                        rhs=es4d_h,
                        start=False,
                        stop=True,
                    )
                    # diag pair (k2,k3): es_prev is the k=2 tile (cols 256:512)
                    nc.vector.tensor_add(
                        out=es_prev[:, c3:SB], in0=es_prev[:, c3:SB], in1=es4d_h
                    )
                    nc.tensor.matmul(
                        out=ps_den[:, 2 * HD : SB],
                        lhsT=ones_col,
                        rhs=es_prev[:, 2 * HD : SB],
                        start=False,
                        stop=True,
                    )
                    # normalize: on = ps_o * (1/den) broadcast over partitions
                    rb1 = work.tile([1, SB], F32, tag="rb1")
                    nc.vector.reciprocal_approx_fast(out=rb1, in_=ps_den)
                    db = work.tile([HD, SB], F32, tag="db")
                    nc.gpsimd.partition_broadcast(db, rb1, channels=HD)
                    nc.vector.tensor_mul(
                        out=on_all[:, h, s0 : s0 + SB], in0=ps_o, in1=db
                    )

                # ---- wo for this s-block: y[t, :] = sum_h on_h^T @ wo_h ----
                # wo psum shares the "o" ring (attention AV is done for sj)
                for st in range(4):
                    t0 = s0 + st * HD
                    y_sb = work.tile([HD, NSJ, SB], FP16, tag="y_sb", bufs=2)
                    for dj in range(NSJ):
                        ps_y = ps.tile(
                            [HD, SB], F32, tag="o", bufs=2, name=f"y_{sj}_{st}_{dj}"
                        )
                        for hh in range(NH):
                            nc.tensor.matmul(
                                out=ps_y,
                                lhsT=on_all[:, hh, t0 : t0 + HD],
                                rhs=wo_sb[:, hh, dj * SB : (dj + 1) * SB],
                                start=(hh == 0),
                                stop=(hh == NH - 1),
                            )
                        if dj % 2 == 0:
                            nc.scalar.copy(out=y_sb[:, dj, :], in_=ps_y)
                        else:
                            nc.vector.tensor_copy(out=y_sb[:, dj, :], in_=ps_y)
                    nc.sync.dma_start(out=y_d[t0 : t0 + HD, :], in_=y_sb)

    nc.compile()
    return nc


def _get_program():
    global _PROG
    if _PROG is None:
        _PROG = _build_program()
    return _PROG


def _make_in_maps(x, freqs_cos, freqs_sin, wq, wk, wv, wo):
    perm = np.concatenate([np.arange(0, HD, 2), np.arange(1, HD, 2)])  # even|odd

    cs = np.ascontiguousarray(np.asarray(freqs_cos, np.float32).T).astype(F16)
    sc = np.ascontiguousarray(np.asarray(freqs_sin, np.float32).T).astype(F16)

    # triangular causal mask, replicated x4 for the fused diagonal block
    tt = np.arange(HD)[:, None]
    ss = np.arange(HD)[None, :]
    tri = (tt <= ss).astype(np.float32)  # [128, 128]
    mask4 = np.tile(tri, (1, NH)).astype(F16)  # [128, 4*128]
    ident = np.eye(HD, dtype=np.float32).astype(F16)
    ones = np.ones((HD, HD), dtype=np.float32).astype(F16)

    # permute q/k head-dim columns so rope pairs land on partition halves
    def permute_heads(w, n_heads):
        w = w.reshape(D, n_heads, HD)
        return w[:, :, perm].reshape(D, n_heads * HD)

    wq_p = permute_heads(np.asarray(wq, np.float32), N_HEADS)
    wk_p = permute_heads(np.asarray(wk, np.float32), N_KV_HEADS)
    wv_ = np.asarray(wv, np.float32)
    wo_ = np.asarray(wo, np.float32)

    xt16 = [
        np.ascontiguousarray(np.asarray(x, np.float32)[b].T).astype(F16)
        for b in range(B)
    ]

    in_maps = []
    for c in range(8):
        b, g = divmod(c, 4)
        wqkv = np.concatenate(
            [
                wk_p[:, g * HD : (g + 1) * HD],
                wv_[:, g * HD : (g + 1) * HD],
                wq_p[:, g * NH * HD : (g + 1) * NH * HD],
            ],
            axis=1,
        )
        # rearrange to [p, m, t, c] so each m-slice is 4KB/partition contiguous
        wqkv = (
            wqkv.reshape(NT, HD, NH + 2, HD)
            .transpose(1, 2, 0, 3)
            .reshape(HD, (NH + 2) * NT * HD)
        )
        wqkv = np.ascontiguousarray(wqkv).astype(F16)
        in_maps.append(
            {
                "xt": xt16[b],
                "wqkv": wqkv,
                "wo": np.ascontiguousarray(
                    wo_[g * NH * HD : (g + 1) * NH * HD, :]
                ).astype(F16),
                "cs": cs,
                "sc": sc,
                "mask": mask4,
                "ident": ident,
                "ones": ones,
            }
        )
    return in_maps


def run(x, freqs_cos, freqs_sin, wq, wk, wv, wo, trace=False):
    from concourse.bass_utils import run_bass_kernel_spmd

    nc = _get_program()
    in_maps = _make_in_maps(x, freqs_cos, freqs_sin, wq, wk, wv, wo)
    res = run_bass_kernel_spmd(nc, in_maps, list(range(8)), trace=trace)
    out = np.empty((B, S, D), dtype=np.float32)
    for b in range(B):
        acc = res.results[b * 4]["y"].astype(np.float32)
        for g in range(1, 4):
            acc += res.results[b * 4 + g]["y"].astype(np.float32)
        out[b] = acc
    return out, res


def kernel(x, freqs_cos, freqs_sin, wq, wk, wv, wo):
    out, _ = run(x, freqs_cos, freqs_sin, wq, wk, wv, wo, trace=False)
    return out


# revision 29
# speedup vs baseline: 1.3959x; 1.1643x over previous
"""GQA attention forward (B=2, S=2048, D=2048, 16 q heads / 4 kv heads, RoPE,
causal) on 8 Trainium2 NeuronCores.

Sharding: core c <-> (batch b = c//4, kv-group g = c%4). Each core computes its
4 query heads + 1 kv head end-to-end, including its row-shard of wo; the host
sums the 4 wo-partials per batch (the "all-reduce after wo" of the tensor
parallel scheme, done at gather time).

v2 layout/schedule:
  - everything stored fp16 (weights, q/k/v, es, on, wo, y partial): halves DMA
    and SBUF, speeds PE weight loads, enables 2x DVE modes; psum stays f32.
  - x is passed transposed (d-major); wq/wk columns permuted per head so RoPE
    is elementwise on partition halves (permutation cancels in q.k).
  - RoPE packed as 2 full-width muls + 2 half-width add/sub via [cos;sin] and
    [sin;cos] stacked constants.
  - scores built transposed ([t, s]); softmax denominator via ones-column
    matmul; no max-subtraction (scores are O(1), exp safe, shift-invariant).
  - the 4 heads' k=3 diagonal score tiles fused into one 512-moving matmul.
  - attention output `on` stays SBUF-resident; wo is interleaved per s-block
    (no DRAM spill, no serial tail).
  - normalization: DVE reciprocal from psum + gpsimd partition broadcast.
"""

import ml_dtypes
import numpy as np

F16 = ml_dtypes.float16 if hasattr(ml_dtypes, "float16") else np.float16
B, S, D = 2, 2048, 2048
N_HEADS, N_KV_HEADS, HD = 16, 4, 128
NH = N_HEADS // N_KV_HEADS  # q heads per core = 4
SB = 512                    # s-block (moving dim per matmul)
NSJ = S // SB               # 4 s-blocks
NT = S // HD                # 16 t-tiles (and d-tiles)
SCALE = 1.0 / np.sqrt(HD).astype(np.float32)

_PROG = None  # built once per process


def _build_program():
    import concourse.bacc as bacc
    import concourse.tile as tile
    from concourse import bass_isa, mybir

    F32 = mybir.dt.float32
    FP16 = mybir.dt.float16
    Exp = mybir.ActivationFunctionType.Exp

    nc = bacc.Bacc("TRN2", target_bir_lowering=False, debug=False)

    xt_d = nc.declare_dram_parameter("xt", [D, S], FP16, isOutput=False)
    # wqkv pre-arranged host-side as [p, m, t, c] so each m-slice DMA reads
    # 4KB contiguous per partition
    wqkv_d = nc.declare_dram_parameter(
        "wqkv", [HD, (NH + 2) * NT * HD], FP16, isOutput=False
    )
    wo_d = nc.declare_dram_parameter("wo", [NH * HD, D], FP16, isOutput=False)
    cs_d = nc.declare_dram_parameter("cs", [HD // 2, S], FP16, isOutput=False)  # cos^T
    sc_d = nc.declare_dram_parameter("sc", [HD // 2, S], FP16, isOutput=False)  # sin^T
    mask_d = nc.declare_dram_parameter("mask", [HD, NH * HD], FP16, isOutput=False)
    ident_d = nc.declare_dram_parameter("ident", [HD, HD], FP16, isOutput=False)
    ones_d = nc.declare_dram_parameter("ones", [HD, HD], FP16, isOutput=False)
    y_d = nc.declare_dram_parameter("y", [S, D], FP16, isOutput=True)

    NM = NH + 2  # 6 projection column-blocks: k, v, q0..q3
    H2 = HD // 2

    with tile.TileContext(nc) as tc:
        with (
            tc.tile_pool(name="consts", bufs=1) as consts,
            tc.tile_pool(name="persist", bufs=1) as persist,
            tc.tile_pool(name="work", bufs=2) as work,
            tc.tile_pool(name="ps", bufs=1, space="PSUM") as ps,
        ):
            # ---- small constants ----
            mask4 = consts.tile([HD, NH * HD], FP16, tag="mask4")  # tri mask x4
            ident = consts.tile([HD, HD], FP16, tag="ident")
            ones_sb = consts.tile([HD, HD], FP16, tag="ones_sb")
            ones_col = ones_sb[:, 0:1]
            cs_sb = consts.tile([HD // 2, S], FP16, tag="cs_sb")
            sc_sb = consts.tile([HD // 2, S], FP16, tag="sc_sb")

            # ---- persistent SBUF tensors ----
            kt = persist.tile([HD, S], FP16, tag="kt")            # k^T rope'd
            v_sb = persist.tile([HD, NT, HD], FP16, tag="v_sb")   # v[t, hd]
            wqkv = persist.tile([HD, NT, NM * HD], FP16, tag="wqkv")
            wo_sb = persist.tile([HD, NH, D], FP16, tag="wo_sb")
            on_all = persist.tile([HD, NH, S], FP16, tag="on_all")

            xt_r = xt_d[:, :].rearrange("(t p) s -> p t s", p=HD)
            wqkv_r = wqkv_d[:, :].rearrange("p (m t c) -> p m t c", m=NM, t=NT)

            # critical-path loads first: wqkv m-slice 0 (split so the first
            # contraction tiles land sooner) + first x block, then remaining
            # m-slices in consumption order
            nc.scalar.dma_start(
                out=wqkv[:, 0:4, 0:HD], in_=wqkv_r[:, 0, 0:4, :]
            )
            nc.scalar.dma_start(
                out=wqkv[:, 4:NT, 0:HD], in_=wqkv_r[:, 0, 4:NT, :]
            )
            sj0_quarters = []
            for ck in range(4):
                xq = work.tile(
                    [HD, NT // 4, SB], FP16, tag="xts", bufs=8, name=f"xts_0_{ck}"
                )
                nc.sync.dma_start(out=xq, in_=xt_r[:, ck * 4 : (ck + 1) * 4, 0:SB])
                sj0_quarters.append(xq)
            nc.gpsimd.dma_start(out=cs_sb, in_=cs_d[:, :])
            nc.gpsimd.dma_start(out=sc_sb, in_=sc_d[:, :])
            for m in range(1, NM):
                nc.scalar.dma_start(
                    out=wqkv[:, :, m * HD : (m + 1) * HD], in_=wqkv_r[:, m, :, :]
                )
            nc.gpsimd.dma_start(out=mask4, in_=mask_d[:, :])
            nc.gpsimd.dma_start(out=ident, in_=ident_d[:, :])
            nc.gpsimd.dma_start(out=ones_sb, in_=ones_d[:, :])
            nc.scalar.dma_start(
                out=wo_sb, in_=wo_d[:, :].rearrange("(h p) d -> p h d", p=HD)
            )

            for sj in range(NSJ):
                s0 = sj * SB
                # ---- projection of x^T[:, s-block] ----
                if sj == 0:
                    quarters = sj0_quarters
                else:
                    quarters = []
                    for ck in range(4):
                        xq = work.tile(
                            [HD, NT // 4, SB], FP16, tag="xts", bufs=8,
                            name=f"xts_{sj}_{ck}",
                        )
                        nc.sync.dma_start(
                            out=xq, in_=xt_r[:, ck * 4 : (ck + 1) * 4, s0 : s0 + SB]
                        )
                        quarters.append(xq)
                q4 = work.tile([HD, NH, SB], FP16, tag="q4", bufs=2, name=f"q4_{sj}")
                for m in range(NM):  # 0=k, 1=v, 2..5=q heads
                    pp = ps.tile([HD, SB], F32, tag="pp", bufs=2)
                    for dt in range(NT):
                        nc.tensor.matmul(
                            out=pp,
                            lhsT=wqkv[:, dt, m * HD : (m + 1) * HD],
                            rhs=quarters[dt // 4][:, dt % 4, :],
                            start=(dt == 0),
                            stop=(dt == NT - 1),
                        )
                    if m != 1:
                        # rope: rows 0:64 = even dims (xr), 64:128 = odd (xi)
                        if m == 0:
                            dst = kt[:, s0 : s0 + SB]
                        else:
                            dst = q4[:, m - 2, :]
                        c = cs_sb[:, s0 : s0 + SB]
                        sn = sc_sb[:, s0 : s0 + SB]
                        ta = work.tile([H2, SB], FP16, tag="rope_a")
                        tb = work.tile([H2, SB], FP16, tag="rope_b")
                        nc.vector.tensor_mul(out=ta, in0=pp[0:H2, :], in1=c)
                        nc.vector.tensor_mul(out=tb, in0=pp[H2:HD, :], in1=sn)
                        nc.vector.tensor_sub(out=dst[0:H2, :], in0=ta, in1=tb)
                        tc2 = work.tile([H2, SB], FP16, tag="rope_a")
                        td = work.tile([H2, SB], FP16, tag="rope_b")
                        nc.vector.tensor_mul(out=tc2, in0=pp[0:H2, :], in1=sn)
                        nc.vector.tensor_mul(out=td, in0=pp[H2:HD, :], in1=c)
                        nc.vector.tensor_add(out=dst[H2:HD, :], in0=tc2, in1=td)
                    else:
                        # v: psum holds v^T[hd, s-block]; transpose to v[t, hd]
                        vt_sb = work.tile([HD, SB], FP16, tag="vt_sb")
                        nc.scalar.copy(out=vt_sb, in_=pp)
                        for q in range(SB // HD):
                            pt = ps.tile([HD, HD], FP16, tag="s", bufs=3)
                            nc.tensor.transpose(
                                pt, vt_sb[:, q * HD : (q + 1) * HD], ident
                            )
                            nc.scalar.copy(out=v_sb[:, sj * 4 + q, :], in_=pt)

                # ---- fused diagonal (k=3) scores for all 4 heads ----
                ti3 = 4 * sj + 3
                ps_sd = ps.tile([HD, SB], F32, tag="s", bufs=3, name=f"sd_{sj}")
                nc.tensor.matmul(
                    out=ps_sd,
                    lhsT=kt[:, ti3 * HD : (ti3 + 1) * HD],
                    rhs=q4[:, :, 3 * HD : SB],
                    start=True,
                    stop=True,
                )
                es4d = work.tile([HD, NH * HD], FP16, tag="es4d", bufs=2)
                nc.scalar.activation(
                    out=es4d, in_=ps_sd, func=Exp, scale=float(SCALE)
                )
                nc.vector.tensor_mul(out=es4d, in0=es4d, in1=mask4)

                # ---- attention for all heads at this s-block ----
                # softmax denominator: es tiles are pair-summed on the DVE and
                # the ones-matmul runs once per pair (halves the PE den rows)
                nt = 4 * sj + 4  # causal: t-tiles 0..nt-1
                for h in range(NH):
                    ps_o = ps.tile([HD, SB], F32, tag="o", bufs=2, name=f"o_{sj}_{h}")
                    ps_den = ps.tile(
                        [1, SB], F32, tag="den", bufs=1, name=f"d_{sj}_{h}"
                    )
                    nfull = 4 * sj  # full tiles (k < 0)
                    npairs = nfull // 2 + 2  # + 2 diagonal pairs
                    pair_idx = 0
                    es_prev = None
                    for ti in range(nt - 1):
                        k = ti - 4 * sj
                        c0 = max(0, k) * HD  # first valid column (diag band)
                        ps_s = ps.tile(
                            [HD, SB], F32, tag="s", bufs=3, name=f"s_{sj}_{h}_{ti}"
                        )
                        nc.tensor.matmul(
                            out=ps_s[:, c0:SB],
                            lhsT=kt[:, ti * HD : (ti + 1) * HD],
                            rhs=q4[:, h, c0:SB],
                            start=True,
                            stop=True,
                        )
                        es = work.tile([HD, SB], FP16, tag="es", bufs=4)
                        nc.scalar.activation(
                            out=es[:, c0:SB], in_=ps_s[:, c0:SB], func=Exp,
                            scale=float(SCALE),
                        )
                        if k >= 0:
                            # triangular part: first 128 valid columns
                            nc.vector.tensor_mul(
                                out=es[:, c0 : c0 + HD],
                                in0=es[:, c0 : c0 + HD],
                                in1=mask4[:, 0:HD],
                            )
                        nc.tensor.matmul(
                            out=ps_o[:, c0:SB],
                            lhsT=v_sb[:, ti, :],
                            rhs=es[:, c0:SB],
                            start=(ti == 0),
                            stop=False,
                        )
                        if k < 0 or k == 2:
                            if es_prev is None:
                                es_prev = es
                            else:
                                # full-tile pair: sum into even tile, one den mm
                                nc.vector.tensor_add(
                                    out=es_prev, in0=es_prev, in1=es
                                )
                                nc.tensor.matmul(
                                    out=ps_den,
                                    lhsT=ones_col,
                                    rhs=es_prev,
                                    start=(pair_idx == 0),
                                    stop=False,
                                )
                                pair_idx += 1
                                es_prev = None
                        elif k == 0:
                            es_k0 = es
                        elif k == 1:
                            # diag pair (k0,k1): k1 covers cols 128:512
                            nc.vector.tensor_add(
                                out=es_k0[:, HD:SB],
                                in0=es_k0[:, HD:SB],
                                in1=es[:, HD:SB],
                            )
                            nc.tensor.matmul(
                                out=ps_den,
                                lhsT=ones_col,
                                rhs=es_k0,
                                start=(pair_idx == 0),
                                stop=False,
                            )
                            pair_idx += 1
                    # diagonal tile k=3 from the fused block
                    c3 = 3 * HD
                    es4d_h = es4d[:, h * HD : (h + 1) * HD]
                    nc.tensor.matmul(
                        out=ps_o[:, c3:SB],
                        lhsT=v_sb[:, ti3, :],
                        rhs=es4d_h,
                        start=False,
                        stop=True,
                    )
                    # diag pair (k2,k3): es_prev is the k=2 tile (cols 256:512)
                    nc.vector.tensor_add(
                        out=es_prev[:, c3:SB], in0=es_prev[:, c3:SB], in1=es4d_h
                    )
                    nc.tensor.matmul(
                        out=ps_den[:, 2 * HD : SB],
                        lhsT=ones_col,
                        rhs=es_prev[:, 2 * HD : SB],
                        start=False,
                        stop=True,
                    )
                    # normalize: on = ps_o * (1/den) broadcast over partitions
                    rb1 = work.tile([1, SB], F32, tag="rb1")
                    nc.vector.reciprocal_approx_fast(out=rb1, in_=ps_den)
                    db = work.tile([HD, SB], F32, tag="db")
                    nc.gpsimd.partition_broadcast(db, rb1, channels=HD)
                    nc.vector.tensor_mul(
                        out=on_all[:, h, s0 : s0 + SB], in0=ps_o, in1=db
                    )

                # ---- wo, deferred one s-block so wo_sb's load and the wo
                # matmuls hide under the next block's projection/attention ----
                sjw = sj - 1 if sj > 0 else None
                wo_blocks = [sjw] if sjw is not None else []
                if sj == NSJ - 1:
                    wo_blocks.append(sj)
                for sjb in wo_blocks:
                    sw0 = sjb * SB
                    for st in range(4):
                        t0 = sw0 + st * HD
                        y_sb = work.tile([HD, NSJ, SB], FP16, tag="y_sb", bufs=2)
                        for dj in range(NSJ):
                            ps_y = ps.tile(
                                [HD, SB], F32, tag="o", bufs=2,
                                name=f"y_{sjb}_{st}_{dj}",
                            )
                            for hh in range(NH):
                                nc.tensor.matmul(
                                    out=ps_y,
                                    lhsT=on_all[:, hh, t0 : t0 + HD],
                                    rhs=wo_sb[:, hh, dj * SB : (dj + 1) * SB],
                                    start=(hh == 0),
                                    stop=(hh == NH - 1),
                                )
                            if dj % 2 == 0:
                                nc.scalar.copy(out=y_sb[:, dj, :], in_=ps_y)
                            else:
                                nc.vector.tensor_copy(out=y_sb[:, dj, :], in_=ps_y)
                        if st % 2 == 0:
                            nc.sync.dma_start(out=y_d[t0 : t0 + HD, :], in_=y_sb)
                        else:
                            nc.scalar.dma_start(out=y_d[t0 : t0 + HD, :], in_=y_sb)

    nc.compile()
    return nc


def _get_program():
    global _PROG
    if _PROG is None:
        _PROG = _build_program()
    return _PROG


def _make_in_maps(x, freqs_cos, freqs_sin, wq, wk, wv, wo):
    perm = np.concatenate([np.arange(0, HD, 2), np.arange(1, HD, 2)])  # even|odd

    cs = np.ascontiguousarray(np.asarray(freqs_cos, np.float32).T).astype(F16)
    sc = np.ascontiguousarray(np.asarray(freqs_sin, np.float32).T).astype(F16)

    # triangular causal mask, replicated x4 for the fused diagonal block
    tt = np.arange(HD)[:, None]
    ss = np.arange(HD)[None, :]
    tri = (tt <= ss).astype(np.float32)  # [128, 128]
    mask4 = np.tile(tri, (1, NH)).astype(F16)  # [128, 4*128]
    ident = np.eye(HD, dtype=np.float32).astype(F16)
    ones = np.ones((HD, HD), dtype=np.float32).astype(F16)

    # permute q/k head-dim columns so rope pairs land on partition halves
    def permute_heads(w, n_heads):
        w = w.reshape(D, n_heads, HD)
        return w[:, :, perm].reshape(D, n_heads * HD)

    wq_p = permute_heads(np.asarray(wq, np.float32), N_HEADS)
    wk_p = permute_heads(np.asarray(wk, np.float32), N_KV_HEADS)
    wv_ = np.asarray(wv, np.float32)
    wo_ = np.asarray(wo, np.float32)

    xt16 = [
        np.ascontiguousarray(np.asarray(x, np.float32)[b].T).astype(F16)
        for b in range(B)
    ]

    in_maps = []
    for c in range(8):
        b, g = divmod(c, 4)
        wqkv = np.concatenate(
            [
                wk_p[:, g * HD : (g + 1) * HD],
                wv_[:, g * HD : (g + 1) * HD],
                wq_p[:, g * NH * HD : (g + 1) * NH * HD],
            ],
            axis=1,
        )
        # rearrange to [p, m, t, c] so each m-slice is 4KB/partition contiguous
        wqkv = (
            wqkv.reshape(NT, HD, NH + 2, HD)
            .transpose(1, 2, 0, 3)
            .reshape(HD, (NH + 2) * NT * HD)
        )
        wqkv = np.ascontiguousarray(wqkv).astype(F16)
        in_maps.append(
            {
                "xt": xt16[b],
                "wqkv": wqkv,
                "wo": np.ascontiguousarray(
                    wo_[g * NH * HD : (g + 1) * NH * HD, :]
                ).astype(F16),
                "cs": cs,
                "sc": sc,
                "mask": mask4,
                "ident": ident,
                "ones": ones,
            }
        )
    return in_maps


def run(x, freqs_cos, freqs_sin, wq, wk, wv, wo, trace=False):
    from concourse.bass_utils import run_bass_kernel_spmd

    nc = _get_program()
    in_maps = _make_in_maps(x, freqs_cos, freqs_sin, wq, wk, wv, wo)
    res = run_bass_kernel_spmd(nc, in_maps, list(range(8)), trace=trace)
    out = np.empty((B, S, D), dtype=np.float32)
    for b in range(B):
        acc = res.results[b * 4]["y"].astype(np.float32)
        for g in range(1, 4):
            acc += res.results[b * 4 + g]["y"].astype(np.float32)
        out[b] = acc
    return out, res


def kernel(x, freqs_cos, freqs_sin, wq, wk, wv, wo):
    out, _ = run(x, freqs_cos, freqs_sin, wq, wk, wv, wo, trace=False)
    return out
